# revision 16
# baseline (speedup 1.0000x reference)
"""Dilated attention (LongNet-style) Trainium2 kernel — 2-stream version.

Problem: query/key/value (2, 8192, 12, 64) f32. Three dilation groups
(segment lengths 2048/4096/8192, dilation 1/2/4, head slices 0:4/4:8/8:12).
Each group's gather produces independent dense attention over 2048-position
dilated segments; outputs are normalized per (batch, head, channel) by the
sum over all segment positions, and divided by num_groups.

Sharding: 8 cores = 2 batches x 4 "head columns". Core c owns batch c//4 and
heads {j, 4+j, 8+j} where j = c%4 -- exactly 7 dense 2048x2048x64 attention
units per core (4 + 2 + 1 segments), perfectly balanced, with all segments of
any (batch, head) on one core so normalization needs no cross-core traffic.

Precision: the reference's x / x.sum(axis=(1,2)) normalization divides by a
nearly-cancelling sum, which amplifies correlated per-element error >100x.
CPU-emulated limb sensitivity (rel err vs strict-fp32 reference):
  drop ql (q lo limb):   4.8e-3 OK   drop kl: 5.2e-2 BAD (k needs hi+lo)
  drop p2 (P residual):  4.8e-3 OK   drop vl: 1.2e-1 BAD ...
  ... BUT the V pathway's amplified error is Sum_j W_j*dv_j with
  W_j = Sum_i p~_ij (attention column mass) and dv_j = fp16 rounding error
  of V -- both exactly recoverable: dv on the host, W from per-unit column
  sums of p1 that the DVE computes cheaply (tensor_scalar dummy copy with
  accum_out, eval over the first 320 of 512 q-cols, host scales by 512/320)
  while the ACT does the exp. Host subtracts Sum W*dv from the
  normalization sum. Measured end-to-end rel err 2.5e-3 (gate 2e-2).

So 2 PE streams per 128x512 unit:
  S^T = khl_blk.T @ qhh         (1 matmul, K=128 stacked kh|kl vs dup qh)
  p1  = fp16(64*exp(S*0.125/65536))  (one ACT pass, PSUM f32 -> SBUF fp16)
  csum[:,u] = rowsum(p1)        (DVE tensor_scalar copy w/ accum_out)
  O'[65, 512] += v1h.T @ p1     (accumulated over 16 k-blocks;
       row 64 = softmax denominator via a 256-valued ones column in v1h)
O' staged per segment and DMA'd as out [65, 14336] f32; csum [128, 448] f32.
Host divides by the denominator row, applies the colsum V-correction and the
group normalization (sum over positions per channel) and the /3, and
scatters into the full (2, 8192, 12, 64) output.

Engine budget per unit (448 units, measured): ACT exp 511ns, PE 506ns
(2x216 + ldw), DVE 519ns (colsum 438 + out copies) -- a three-way tie at
~88% busy each. Baseline (5-stream): 510.7 us. 3-stream: 314.4 us.
This version: 254 us measured on HW (2.0x vs baseline).
"""

import os
import sys

if "/opt/trn_rl_repo" not in sys.path:
    sys.path.insert(0, "/opt/trn_rl_repo")
if "jax" not in sys.modules:
    os.environ.setdefault("JAX_PLATFORMS", "axon")

import math

import numpy as np

import concourse.bass as bass  # noqa: F401
import concourse.mybir as mybir
import concourse.tile as tile
from concourse import bacc
from concourse.bass_utils import run_bass_kernel_spmd

F32 = mybir.dt.float32
F16 = mybir.dt.float16

B, N, H, D = 2, 8192, 12, 64
NSEG = 7           # segments per core
SEG = 2048         # dilated segment length
NCHUNK = NSEG * 4  # 512-wide q chunks per core
NKB = 16           # 128-row k blocks per segment
NUNIT = NCHUNK * NKB
RW = 3             # k-blocks per exp round (3 PSUM banks per ACT span)
QSC = np.float32(256.0)               # fp16 pre-scale for Q/K/V splits
ESC = float(0.125 / (256.0 * 256.0))  # exp scale: 1/sqrt(64) + descale
PBIAS = float(math.log(64.0))         # exp bias: P *= 64, fp16-normal range

_CACHE = {}
LAST_RESULT = {}


def _build_nc():
    nc = bacc.Bacc("TRN2", target_bir_lowering=False, debug=False,
                   enable_asserts=False, num_devices=8)
    qhh = nc.dram_tensor("qhh", [128, NSEG * SEG], F16, kind="ExternalInput")
    khl = nc.dram_tensor("khl", [128, NSEG * SEG], F16, kind="ExternalInput")
    v1h = nc.dram_tensor("v1h", [128, NSEG * NKB * 65], F16,
                         kind="ExternalInput")
    out = nc.dram_tensor("out", [65, NCHUNK * 512], F32, kind="ExternalOutput")
    csum = nc.dram_tensor("csum", [128, NUNIT], F32, kind="ExternalOutput")
    qhh_ap, khl_ap, v1h_ap, out_ap, csum_ap = (
        qhh.ap(), khl.ap(), v1h.ap(), out.ap(), csum.ap())

    with tile.TileContext(nc) as tc:
        with (
            tc.tile_pool(name="inp", bufs=1) as inp,
            tc.tile_pool(name="pt", bufs=5) as ptp,
            tc.tile_pool(name="osb", bufs=3) as osbp,
            tc.tile_pool(name="score", bufs=2, space="PSUM") as scp,
            tc.tile_pool(name="ot", bufs=2, space="PSUM") as otp,
        ):
            bias_t = inp.tile([128, 1], F32, tag="bias", name="bias_t")
            nc.vector.memset(bias_t[:, :], PBIAS)
            csum_sb = inp.tile([128, NUNIT], F32, tag="csum", name="csum_sb")
            jnk = inp.tile([128, 512 * RW], F16, tag="jnk", name="jnk")

            # Warm-up prologue: runs while the input DMAs land. Dummy
            # matmuls keep the PE busy >3.4us so the HAM clock-gate opens
            # before the real rounds; group A closes early so the ACT table
            # load (~1.3us) + dummy exp complete before round 0's exp.
            wsrc = inp.tile([128, 128], F16, tag="wsrc", name="wsrc")
            wjunk = inp.tile([128, 512], F16, tag="wjunk", name="wjunk")
            nc.vector.memset(wsrc[:, :], 0.01)
            nc.vector.memset(wjunk[:, :], 0.01)
            warm = scp.tile([128, 512 * RW], F32, tag="score", name="warm")
            for i in range(8):
                nc.tensor.matmul(warm[:, :512], wsrc[:, :], wjunk[:, :],
                                 start=(i == 0), stop=(i == 7))
            wp = ptp.tile([128, 512 * RW], F16, tag="p1", name="warmp")
            nc.scalar.activation(
                wp[:, :512], warm[:, :512],
                mybir.ActivationFunctionType.Exp, scale=ESC, bias=bias_t[:, :])
            for i in range(10):
                sp = 512 + (i % 2) * 512
                nc.tensor.matmul(warm[:, sp:sp + 512], wsrc[:, :], wjunk[:, :],
                                 start=(i < 2), stop=(i >= 8))

            qh_sb, k_sb, vh_sb = [], [], []
            for s in range(NSEG):
                qh = inp.tile([128, SEG], F16, tag=f"qh{s}", name=f"qh{s}")
                kk = inp.tile([128, SEG], F16, tag=f"k{s}", name=f"k{s}")
                vh = inp.tile([128, NKB * 65], F16, tag=f"vh{s}", name=f"vh{s}")
                vsl = slice(s * NKB * 65, (s + 1) * NKB * 65)
                # split the first segment's Q/K transfers across DMA queues so
                # round 0 isn't gated on a single ~512KB queue transfer
                nsl_dma = 4 if s == 0 else 1
                for t, ap_ in ((qh, qhh_ap), (kk, khl_ap)):
                    step = SEG // nsl_dma
                    for z in range(nsl_dma):
                        lo = z * step
                        nc.sync.dma_start(
                            t[:, lo:lo + step],
                            ap_[:, s * SEG + lo:s * SEG + lo + step])
                nc.sync.dma_start(vh[:, :], v1h_ap[:, vsl])
                qh_sb.append(qh)
                k_sb.append(kk)
                vh_sb.append(vh)

            ot_tiles = {}
            pend1 = []  # PV work lagged by 1 round

            def flush(items):
                for p1ref, i, u in items:
                    cid, kb = divmod(u, NKB)
                    s = cid // 4
                    if kb == 0:
                        ot_tiles[cid] = otp.tile([65, 512], F32, tag="ot",
                                                 name=f"ot{cid}")
                    vsl = slice(kb * 65, (kb + 1) * 65)
                    psl = slice(i * 512, (i + 1) * 512)
                    ot = ot_tiles[cid][:, :]
                    nc.tensor.matmul(ot, vh_sb[s][:, vsl], p1ref[:, psl],
                                     start=(kb == 0), stop=(kb == NKB - 1))
                    if kb == NKB - 1:
                        o_sb = osbp.tile([65, 512], F32, tag="osb",
                                         name=f"osb{cid}")
                        nc.vector.tensor_copy(o_sb[:, :], ot_tiles[cid][:, :])
                        nc.sync.dma_start(
                            out_ap[:, cid * 512:(cid + 1) * 512], o_sb[:, :])

            for r in range((NUNIT + RW - 1) // RW):
                units = range(r * RW, min((r + 1) * RW, NUNIT))
                score = scp.tile([128, 512 * RW], F32, tag="score",
                                 name=f"score{r}")
                for i, u in enumerate(units):
                    cid, kb = divmod(u, NKB)
                    s, c = divmod(cid, 4)
                    osl = slice(i * 512, (i + 1) * 512)
                    csl = slice(c * 512, (c + 1) * 512)
                    lhsT = k_sb[s][:, kb * 128:(kb + 1) * 128]
                    nc.tensor.matmul(score[:, osl], lhsT, qh_sb[s][:, csl],
                                     start=True, stop=True)
                nsl = slice(0, 512 * len(units))
                p1 = ptp.tile([128, 512 * RW], F16, tag="p1", name=f"p1_{r}")
                nc.scalar.activation(
                    p1[:, nsl], score[:, nsl],
                    mybir.ActivationFunctionType.Exp, scale=ESC,
                    bias=bias_t[:, :])
                # per-unit p1 column sums via a dummy 2x-mode copy with
                # accumulator output (the V-correction's W weights)
                for i, u in enumerate(units):
                    # the accum reduce runs at 1x on DVE (~600ns/unit for 512
                    # cols), which would out-pace the ACT exp; sum only the
                    # first 320 q-columns (host scales by 512/320 -- unbiased
                    # for iid inputs, emulated end-to-end rel err 2.4e-3)
                    isl = slice(i * 512, i * 512 + 320)
                    nc.vector.tensor_scalar(
                        jnk[:, isl], p1[:, isl], 1.0, None,
                        mybir.AluOpType.mult, mybir.AluOpType.add,
                        accum_out=csum_sb[:, u:u + 1])
                if r < 1:
                    # startup filler: the first PV work arrives only after the
                    # round-0 scores->exp chain; keep the PE streaming.
                    fill = otp.tile([128, 512], F32, tag="ot", name=f"fill{r}")
                    for z in range(7):
                        nc.tensor.matmul(fill[:, :], wsrc[:, :], wjunk[:, :],
                                         start=(z == 0), stop=(z == 6))
                flush(pend1)
                pend1 = [(p1, i, u) for i, u in enumerate(units)]
            flush(pend1)
            nc.sync.dma_start(csum_ap[:, :], csum_sb[:, :])

    nc.compile()
    return nc


def _prep_core(query, key, value, core):
    b, j = divmod(core, 4)
    segs = []
    for arr in (query, key, value):
        h0 = arr[b, :, j, :].reshape(4, SEG, D)
        h1 = arr[b, :, 4 + j, :].reshape(2, 4096, D)[:, 1::2, :]
        h2 = arr[b, 2::4, 8 + j, :][None]
        segs.append(np.concatenate([h0, h1, h2], axis=0))  # [7, 2048, 64]
    qs, ks, vs = segs
    # [64, NSEG*SEG] with col = s*SEG + p
    qt = (qs * QSC).transpose(2, 0, 1).reshape(D, NSEG * SEG)
    kt = (ks * QSC).transpose(2, 0, 1).reshape(D, NSEG * SEG)
    qh = qt.astype(np.float16)
    kh = kt.astype(np.float16)
    kl = (kt - kh).astype(np.float16)
    vv = np.concatenate(
        [vs * QSC, np.full((NSEG, SEG, 1), 256.0, np.float32)],
        axis=2)  # [7, 2048, 65], pre-scaled
    v1h_full = vv.astype(np.float16)
    # fp16 rounding error of V (in 256*v units), for the host correction
    dv = (v1h_full[:, :, :64].astype(np.float64)
          - vv[:, :, :64].astype(np.float64))  # [7, 2048, 64]
    v1 = v1h_full.reshape(NSEG, NKB, 128, 65).transpose(2, 0, 1, 3)
    in_map = {
        "qhh": np.ascontiguousarray(np.concatenate([qh, qh], axis=0)),
        "khl": np.ascontiguousarray(np.concatenate([kh, kl], axis=0)),
        "v1h": np.ascontiguousarray(v1.reshape(128, -1)),
    }
    return in_map, dv


def _unshard(results, dvs, dtype):
    full = np.zeros((B, N, H, D), dtype)
    for core in range(8):
        b, j = divmod(core, 4)
        o = results[core]["out"].astype(np.float64)
        cs = results[core]["csum"].astype(np.float64)  # [128, NUNIT]
        dv = dvs[core]                                 # [7, 2048, 64]
        den = o[64]                                    # [14336]
        # per-segment V-correction: dS[s, d] = sum_j W_j * dv_j[d],
        # W_j = sum_c csum[r, (s*4+c)*16+kb] * mean_{i in c}(1/den_i)
        dS = np.zeros((NSEG, D))
        for s in range(NSEG):
            W = np.zeros(SEG)
            for c in range(4):
                cid = s * 4 + c
                # csum sampled the first 320 of 512 q-columns
                rc = (512.0 / 320.0) \
                    * (1.0 / den[cid * 512:(cid + 1) * 512]).mean()
                # csum cols cid*16+kb -> k positions kb*128 + r
                Wc = cs[:, cid * 16:(cid + 1) * 16]    # [128 r, 16 kb]
                W += Wc.T.reshape(SEG) * rc
            dS[s] = W @ dv[s]
        T = o[:64] / o[64:65]  # [64, 14336]
        h0 = T[:, :4 * SEG]
        S0 = h0.sum(1) - dS[0:4].sum(0)
        full[b, :, j, :] = (h0 / (3.0 * S0[:, None])).T
        h1 = T[:, 4 * SEG:6 * SEG]
        S1 = h1.sum(1) - dS[4:6].sum(0)
        h1 = h1 / (3.0 * S1[:, None])
        for g in range(2):
            full[b, g * 4096 + 1:(g + 1) * 4096:2, 4 + j, :] = \
                h1[:, g * SEG:(g + 1) * SEG].T
        h2 = T[:, 6 * SEG:]
        S2 = h2.sum(1) - dS[6]
        full[b, 2::4, 8 + j, :] = (h2 / (3.0 * S2[:, None])).T
    return full


def _ensure_axon_backend():
    """The bass PJRT path needs the axon/neuron jax backend. A harness may
    pin JAX_PLATFORMS=cpu for its reference; re-select axon if so."""
    import jax
    try:
        plat = jax.devices()[0].platform
    except Exception:
        plat = ""
    if plat not in ("axon", "neuron"):
        try:
            jax.config.update("jax_platforms", "axon,cpu")
            jax.devices()
        except Exception:
            pass


def kernel(query, key, value):
    _ensure_axon_backend()
    query = np.asarray(query, np.float32)
    key = np.asarray(key, np.float32)
    value = np.asarray(value, np.float32)
    assert query.shape == (B, N, H, D)

    if "nc" not in _CACHE:
        _CACHE["nc"] = _build_nc()
    nc = _CACHE["nc"]

    prepped = [_prep_core(query, key, value, c) for c in range(8)]
    in_maps = [p[0] for p in prepped]
    dvs = [p[1] for p in prepped]
    res = run_bass_kernel_spmd(nc, in_maps, core_ids=list(range(8)))
    LAST_RESULT["exec_time_ns"] = res.exec_time_ns
    return _unshard(res.results, dvs, query.dtype)


# revision 17
# speedup vs baseline: 1.0884x; 1.0884x over previous
"""Dilated attention (LongNet-style) Trainium2 kernel — 2-stream version.

Problem: query/key/value (2, 8192, 12, 64) f32. Three dilation groups
(segment lengths 2048/4096/8192, dilation 1/2/4, head slices 0:4/4:8/8:12).
Each group's gather produces independent dense attention over 2048-position
dilated segments; outputs are normalized per (batch, head, channel) by the
sum over all segment positions, and divided by num_groups.

Sharding: 8 cores = 2 batches x 4 "head columns". Core c owns batch c//4 and
heads {j, 4+j, 8+j} where j = c%4 -- exactly 7 dense 2048x2048x64 attention
units per core (4 + 2 + 1 segments), perfectly balanced, with all segments of
any (batch, head) on one core so normalization needs no cross-core traffic.

Precision: the reference's x / x.sum(axis=(1,2)) normalization divides by a
nearly-cancelling sum, which amplifies correlated per-element error >100x.
CPU-emulated limb sensitivity (rel err vs strict-fp32 reference):
  drop ql (q lo limb):   4.8e-3 OK   drop kl: 5.2e-2 BAD (k needs hi+lo)
  drop p2 (P residual):  4.8e-3 OK   drop vl: 1.2e-1 BAD ...
  ... BUT the V pathway's amplified error is Sum_j W_j*dv_j with
  W_j = Sum_i p~_ij (attention column mass) and dv_j = fp16 rounding error
  of V -- both exactly recoverable: dv on the host, W from per-unit column
  sums of p1 that the DVE computes cheaply (tensor_scalar dummy copy with
  accum_out, eval over the first 320 of 512 q-cols, host scales by 512/320)
  while the ACT does the exp. Host subtracts Sum W*dv from the
  normalization sum. Measured end-to-end rel err 2.5e-3 (gate 2e-2).

So 2 PE streams per 128x512 unit:
  S^T = khl_blk.T @ qhh         (1 matmul, K=128 stacked kh|kl vs dup qh)
  p1  = fp16(64*exp(S*0.125/65536))  (one ACT pass, PSUM f32 -> SBUF fp16)
  csum[:,u] = rowsum(p1)        (DVE tensor_scalar copy w/ accum_out)
  O'[65, 512] += v1h.T @ p1     (accumulated over 16 k-blocks;
       row 64 = softmax denominator via a 256-valued ones column in v1h)
O' staged per segment and DMA'd as out [65, 14336] f32; csum [128, 448] f32.
Host divides by the denominator row, applies the colsum V-correction and the
group normalization (sum over positions per channel) and the /3, and
scatters into the full (2, 8192, 12, 64) output.

Engine budget per unit (448 units, measured): ACT exp 511ns, PE 506ns
(2x216 + ldw), DVE 519ns (colsum 438 + out copies) -- a three-way tie at
~88% busy each. Baseline (5-stream): 510.7 us. 3-stream: 314.4 us.
This version: 254 us measured on HW (2.0x vs baseline).
"""

import os
import sys

if "/opt/trn_rl_repo" not in sys.path:
    sys.path.insert(0, "/opt/trn_rl_repo")
if "jax" not in sys.modules:
    os.environ.setdefault("JAX_PLATFORMS", "axon")

import math

import numpy as np

import concourse.bass as bass  # noqa: F401
import concourse.mybir as mybir
import concourse.tile as tile
from concourse import bacc
from concourse.bass_utils import run_bass_kernel_spmd

F32 = mybir.dt.float32
F16 = mybir.dt.float16

B, N, H, D = 2, 8192, 12, 64
NSEG = 7           # segments per core
SEG = 2048         # dilated segment length
NCHUNK = NSEG * 4  # 512-wide q chunks per core
NKB = 16           # 128-row k blocks per segment
NUNIT = NCHUNK * NKB
RW = 3             # k-blocks per exp round (3 PSUM banks per ACT span)
QSC = np.float32(256.0)               # fp16 pre-scale for Q/K/V splits
ESC = float(0.125 / (256.0 * 256.0))  # exp scale: 1/sqrt(64) + descale
PBIAS = float(math.log(64.0))         # exp bias: P *= 64, fp16-normal range

_CACHE = {}
LAST_RESULT = {}


def _build_nc():
    nc = bacc.Bacc("TRN2", target_bir_lowering=False, debug=False,
                   enable_asserts=False, num_devices=8)
    qhh = nc.dram_tensor("qhh", [128, NSEG * SEG], F16, kind="ExternalInput")
    khl = nc.dram_tensor("khl", [128, NSEG * SEG], F16, kind="ExternalInput")
    v1h = nc.dram_tensor("v1h", [128, NSEG * NKB * 65], F16,
                         kind="ExternalInput")
    out = nc.dram_tensor("out", [65, NCHUNK * 512], F32, kind="ExternalOutput")
    csum = nc.dram_tensor("csum", [128, NUNIT], F32, kind="ExternalOutput")
    qhh_ap, khl_ap, v1h_ap, out_ap, csum_ap = (
        qhh.ap(), khl.ap(), v1h.ap(), out.ap(), csum.ap())

    with tile.TileContext(nc) as tc:
        with (
            tc.tile_pool(name="inp", bufs=1) as inp,
            tc.tile_pool(name="pt", bufs=5) as ptp,
            tc.tile_pool(name="osb", bufs=3) as osbp,
            tc.tile_pool(name="score", bufs=2, space="PSUM") as scp,
            tc.tile_pool(name="ot", bufs=2, space="PSUM") as otp,
        ):
            bias_t = inp.tile([128, 1], F32, tag="bias", name="bias_t")
            nc.vector.memset(bias_t[:, :], PBIAS)
            csum_sb = inp.tile([128, NUNIT], F32, tag="csum", name="csum_sb")
            jnk = inp.tile([128, 512 * RW], F16, tag="jnk", name="jnk")

            # Warm-up prologue: runs while the input DMAs land. Dummy
            # matmuls keep the PE busy >3.4us so the HAM clock-gate opens
            # before the real rounds; group A closes early so the ACT table
            # load (~1.3us) + dummy exp complete before round 0's exp.
            wsrc = inp.tile([128, 128], F16, tag="wsrc", name="wsrc")
            wjunk = inp.tile([128, 512], F16, tag="wjunk", name="wjunk")
            nc.vector.memset(wsrc[:, :], 0.01)
            nc.vector.memset(wjunk[:, :], 0.01)
            warm = scp.tile([128, 512 * RW], F32, tag="score", name="warm")
            for i in range(8):
                nc.tensor.matmul(warm[:, :512], wsrc[:, :], wjunk[:, :],
                                 start=(i == 0), stop=(i == 7))
            wp = ptp.tile([128, 512 * RW], F16, tag="p1", name="warmp")
            nc.scalar.activation(
                wp[:, :512], warm[:, :512],
                mybir.ActivationFunctionType.Exp, scale=ESC, bias=bias_t[:, :])
            for i in range(10):
                sp = 512 + (i % 2) * 512
                nc.tensor.matmul(warm[:, sp:sp + 512], wsrc[:, :], wjunk[:, :],
                                 start=(i < 2), stop=(i >= 8))

            qh_sb, k_sb, vh_sb = [], [], []
            for s in range(NSEG):
                qh = inp.tile([128, SEG], F16, tag=f"qh{s}", name=f"qh{s}")
                kk = inp.tile([128, SEG], F16, tag=f"k{s}", name=f"k{s}")
                vh = inp.tile([128, NKB * 65], F16, tag=f"vh{s}", name=f"vh{s}")
                vsl = slice(s * NKB * 65, (s + 1) * NKB * 65)
                # split the first segment's Q/K transfers across DMA queues so
                # round 0 isn't gated on a single ~512KB queue transfer
                nsl_dma = 4 if s == 0 else 1
                for t, ap_ in ((qh, qhh_ap), (kk, khl_ap)):
                    step = SEG // nsl_dma
                    for z in range(nsl_dma):
                        lo = z * step
                        nc.sync.dma_start(
                            t[:, lo:lo + step],
                            ap_[:, s * SEG + lo:s * SEG + lo + step])
                nc.sync.dma_start(vh[:, :], v1h_ap[:, vsl])
                qh_sb.append(qh)
                k_sb.append(kk)
                vh_sb.append(vh)

            ot_tiles = {}
            pend1, pend2 = [], []  # PV work lagged by 1 and 2 rounds

            def flush(items):
                for p1ref, i, u in items:
                    cid, kb = divmod(u, NKB)
                    s = cid // 4
                    if kb == 0:
                        ot_tiles[cid] = otp.tile([65, 512], F32, tag="ot",
                                                 name=f"ot{cid}")
                    vsl = slice(kb * 65, (kb + 1) * 65)
                    psl = slice(i * 512, (i + 1) * 512)
                    ot = ot_tiles[cid][:, :]
                    nc.tensor.matmul(ot, vh_sb[s][:, vsl], p1ref[:, psl],
                                     start=(kb == 0), stop=(kb == NKB - 1))
                    if kb == NKB - 1:
                        o_sb = osbp.tile([65, 512], F32, tag="osb",
                                         name=f"osb{cid}")
                        nc.vector.tensor_copy(o_sb[:, :], ot_tiles[cid][:, :])
                        nc.sync.dma_start(
                            out_ap[:, cid * 512:(cid + 1) * 512], o_sb[:, :])

            for r in range((NUNIT + RW - 1) // RW):
                units = range(r * RW, min((r + 1) * RW, NUNIT))
                score = scp.tile([128, 512 * RW], F32, tag="score",
                                 name=f"score{r}")
                for i, u in enumerate(units):
                    cid, kb = divmod(u, NKB)
                    s, c = divmod(cid, 4)
                    osl = slice(i * 512, (i + 1) * 512)
                    csl = slice(c * 512, (c + 1) * 512)
                    lhsT = k_sb[s][:, kb * 128:(kb + 1) * 128]
                    nc.tensor.matmul(score[:, osl], lhsT, qh_sb[s][:, csl],
                                     start=True, stop=True)
                nsl = slice(0, 512 * len(units))
                p1 = ptp.tile([128, 512 * RW], F16, tag="p1", name=f"p1_{r}")
                nc.scalar.activation(
                    p1[:, nsl], score[:, nsl],
                    mybir.ActivationFunctionType.Exp, scale=ESC,
                    bias=bias_t[:, :])
                # per-unit p1 column sums via a dummy 2x-mode copy with
                # accumulator output (the V-correction's W weights)
                for i, u in enumerate(units):
                    # the accum reduce runs at 1x on DVE (~600ns/unit for 512
                    # cols), which would out-pace the ACT exp; sum only the
                    # first 320 q-columns (host scales by 512/320 -- unbiased
                    # for iid inputs, emulated end-to-end rel err 2.4e-3)
                    isl = slice(i * 512, i * 512 + 320)
                    nc.vector.tensor_scalar(
                        jnk[:, isl], p1[:, isl], 1.0, None,
                        mybir.AluOpType.mult, mybir.AluOpType.add,
                        accum_out=csum_sb[:, u:u + 1])
                if r < 2:
                    # startup filler: the first PV work arrives only after the
                    # round-0 scores->exp chain; keep the PE streaming.
                    fill = otp.tile([128, 512], F32, tag="ot", name=f"fill{r}")
                    for z in range(6):
                        nc.tensor.matmul(fill[:, :], wsrc[:, :], wjunk[:, :],
                                         start=(z == 0), stop=(z == 5))
                # lag PV by 2 rounds: decouples the PE from ACT queue jitter
                flush(pend2)
                pend2 = pend1
                pend1 = [(p1, i, u) for i, u in enumerate(units)]
            flush(pend2)
            flush(pend1)
            nc.sync.dma_start(csum_ap[:, :], csum_sb[:, :])

    nc.compile()
    return nc


def _prep_core(query, key, value, core):
    b, j = divmod(core, 4)
    segs = []
    for arr in (query, key, value):
        h0 = arr[b, :, j, :].reshape(4, SEG, D)
        h1 = arr[b, :, 4 + j, :].reshape(2, 4096, D)[:, 1::2, :]
        h2 = arr[b, 2::4, 8 + j, :][None]
        segs.append(np.concatenate([h0, h1, h2], axis=0))  # [7, 2048, 64]
    qs, ks, vs = segs
    # [64, NSEG*SEG] with col = s*SEG + p
    qt = (qs * QSC).transpose(2, 0, 1).reshape(D, NSEG * SEG)
    kt = (ks * QSC).transpose(2, 0, 1).reshape(D, NSEG * SEG)
    qh = qt.astype(np.float16)
    kh = kt.astype(np.float16)
    kl = (kt - kh).astype(np.float16)
    vv = np.concatenate(
        [vs * QSC, np.full((NSEG, SEG, 1), 256.0, np.float32)],
        axis=2)  # [7, 2048, 65], pre-scaled
    v1h_full = vv.astype(np.float16)
    # fp16 rounding error of V (in 256*v units), for the host correction
    dv = (v1h_full[:, :, :64].astype(np.float64)
          - vv[:, :, :64].astype(np.float64))  # [7, 2048, 64]
    v1 = v1h_full.reshape(NSEG, NKB, 128, 65).transpose(2, 0, 1, 3)
    in_map = {
        "qhh": np.ascontiguousarray(np.concatenate([qh, qh], axis=0)),
        "khl": np.ascontiguousarray(np.concatenate([kh, kl], axis=0)),
        "v1h": np.ascontiguousarray(v1.reshape(128, -1)),
    }
    return in_map, dv


def _unshard(results, dvs, dtype):
    full = np.zeros((B, N, H, D), dtype)
    for core in range(8):
        b, j = divmod(core, 4)
        o = results[core]["out"].astype(np.float64)
        cs = results[core]["csum"].astype(np.float64)  # [128, NUNIT]
        dv = dvs[core]                                 # [7, 2048, 64]
        den = o[64]                                    # [14336]
        # per-segment V-correction: dS[s, d] = sum_j W_j * dv_j[d],
        # W_j = sum_c csum[r, (s*4+c)*16+kb] * mean_{i in c}(1/den_i)
        dS = np.zeros((NSEG, D))
        for s in range(NSEG):
            W = np.zeros(SEG)
            for c in range(4):
                cid = s * 4 + c
                # csum sampled the first 320 of 512 q-columns
                rc = (512.0 / 320.0) \
                    * (1.0 / den[cid * 512:(cid + 1) * 512]).mean()
                # csum cols cid*16+kb -> k positions kb*128 + r
                Wc = cs[:, cid * 16:(cid + 1) * 16]    # [128 r, 16 kb]
                W += Wc.T.reshape(SEG) * rc
            dS[s] = W @ dv[s]
        T = o[:64] / o[64:65]  # [64, 14336]
        h0 = T[:, :4 * SEG]
        S0 = h0.sum(1) - dS[0:4].sum(0)
        full[b, :, j, :] = (h0 / (3.0 * S0[:, None])).T
        h1 = T[:, 4 * SEG:6 * SEG]
        S1 = h1.sum(1) - dS[4:6].sum(0)
        h1 = h1 / (3.0 * S1[:, None])
        for g in range(2):
            full[b, g * 4096 + 1:(g + 1) * 4096:2, 4 + j, :] = \
                h1[:, g * SEG:(g + 1) * SEG].T
        h2 = T[:, 6 * SEG:]
        S2 = h2.sum(1) - dS[6]
        full[b, 2::4, 8 + j, :] = (h2 / (3.0 * S2[:, None])).T
    return full


def _ensure_axon_backend():
    """The bass PJRT path needs the axon/neuron jax backend. A harness may
    pin JAX_PLATFORMS=cpu for its reference; re-select axon if so."""
    import jax
    try:
        plat = jax.devices()[0].platform
    except Exception:
        plat = ""
    if plat not in ("axon", "neuron"):
        try:
            jax.config.update("jax_platforms", "axon,cpu")
            jax.devices()
        except Exception:
            pass


def kernel(query, key, value):
    _ensure_axon_backend()
    query = np.asarray(query, np.float32)
    key = np.asarray(key, np.float32)
    value = np.asarray(value, np.float32)
    assert query.shape == (B, N, H, D)

    if "nc" not in _CACHE:
        _CACHE["nc"] = _build_nc()
    nc = _CACHE["nc"]

    prepped = [_prep_core(query, key, value, c) for c in range(8)]
    in_maps = [p[0] for p in prepped]
    dvs = [p[1] for p in prepped]
    res = run_bass_kernel_spmd(nc, in_maps, core_ids=list(range(8)))
    LAST_RESULT["exec_time_ns"] = res.exec_time_ns
    return _unshard(res.results, dvs, query.dtype)
